# revision 24
# baseline (speedup 1.0000x reference)
"""Multi-head attention (B=2, S=2048, D=1024, H=16, causal mask) on 8 trn2
NeuronCores.

Sharding: 2-way data parallel over batch x 4-way tensor parallel over head
groups (4 heads / core).  Core c handles batch c//4, head group c%4.

Everything on-chip lives feature-major ("transposed") so no transposes are
ever needed: inputs are uploaded chunk-contiguous as x^T [D, S] in fp16;
Q/K projections produce Qh^T/Kh^T [e, t]; scores come out keys-major
[k, q]; exp(p) feeds A@V directly as the moving operand with V (+ a ones
column that makes the softmax denominator fall out of the same matmul)
stationary; the attention output appears as x_att^T [e, q], which is
exactly the layout the output projection wants.  Each core emits its y^T
partial [1024, S] in fp16 and the host sums the 4 partials of each batch
group during unshard (row-parallel TP reduction; on-device collectives are
not launchable as one 8-replica program through this PJRT path).

Schedule: fully chunk-pipelined.  The sequence is processed in 4 query
chunks of 512; chunk qc's attention (scores -> exp -> A@V) is interleaved
on the tensor engine with chunk qc+1's Q/K/V projections and chunk qc-1's
output projection, so the tensor engine never drains while the scalar
engine works through exp, and vice versa.  Softmax runs unnormalized;
normalization happens per head pair as: DVE reciprocal of the denominator
row -> PE ones-matmul broadcast across 64 partitions -> DVE multiply
(no DRAM round trips, no scalar-engine Ln/Exp chains).  Projection bias
adds and V-psum copies run on the scalar engine (idle during attention
bubbles); y tiles convert f32->f16 on DVE and stream out per chunk.

Mask is handled generically: the [S,S] mask is classified on the host into
128x128 blocks (zero / one / mixed).  Zero blocks are skipped entirely
(this is what makes causal cost ~half of dense), mixed blocks get a
pattern-multiply after exp with deduplicated patterns uploaded as data.
"""

import os
import sys

import numpy as np

for _p in ("/opt/trn_rl_repo", "/root/.axon_site/_ro/trn_rl_repo"):
    if os.path.isdir(_p) and _p not in sys.path:
        sys.path.append(_p)

from contextlib import ExitStack  # noqa: E402

import concourse.bass as bass  # noqa: E402
import concourse.tile as tile  # noqa: E402
from concourse import mybir  # noqa: E402

# ----- problem constants (hardcoded per contract) ---------------------------
B, S, D, H, DK = 2, 2048, 1024, 16, 64
NCORES = 8
TP = 4                      # head-parallel ways (per batch group)
EL = D // TP                # 256 local head dims = 4 heads
HL = H // TP                # 4 local heads
QC = 512                    # query-chunk (columns per attention pass)
NQC = S // QC               # 4
KT = 128                    # key tile (contraction tile for A@V)
NKT = S // KT               # 16
P = 128
NMT = D // P                # 8 output-feature tiles
NFT = D // P                # 8 feature (contraction) tiles
SCALE = 1.0 / np.sqrt(DK)

F32 = mybir.dt.float32
if os.environ.get("MHA_BF16", "0") == "1":
    import ml_dtypes
    F16 = mybir.dt.bfloat16
    F16NP = ml_dtypes.bfloat16
else:
    F16 = mybir.dt.float16
    F16NP = np.float16


# ----- host-side mask analysis ---------------------------------------------
class _KTile:
    __slots__ = ("kt", "s0", "s1", "muls", "first", "last")

    def __init__(self, kt, s0, s1, muls):
        self.kt, self.s0, self.s1, self.muls = kt, s0, s1, muls
        self.first = False
        self.last = False


def _mask_plan(mask2d):
    """mask2d: [S, S] ints, mask2d[q, k] (1 = attend).  Returns
    (plan, patterns) where plan[qc] is a list of _KTile and patterns is a
    fp16 array [n_pat, 128, 128] of transposed (k-major) mask blocks."""
    mT = (mask2d != 0).astype(np.float32).T          # [k, q]
    nqt = S // KT
    blk = mT.reshape(NKT, KT, nqt, KT).transpose(0, 2, 1, 3)  # [kt, qt, 128, 128]
    sums = blk.sum(axis=(2, 3))
    patterns = []
    pat_idx = {}

    def pattern_id(kt, qt):
        key = blk[kt, qt].tobytes()
        if key not in pat_idx:
            pat_idx[key] = len(patterns)
            patterns.append(blk[kt, qt].astype(F16NP))
        return pat_idx[key]

    qt_per_qc = QC // KT
    plan = []
    for qc in range(NQC):
        tiles = []
        for kt in range(NKT):
            sub = sums[kt, qc * qt_per_qc:(qc + 1) * qt_per_qc]
            nz = [i for i in range(qt_per_qc) if sub[i] > 0]
            if not nz:
                continue
            s0, s1 = nz[0] * KT, (nz[-1] + 1) * KT
            tiles.append(_KTile(kt, s0, s1, None))
        if not tiles:
            raise ValueError(f"query chunk {qc} has no unmasked keys")
        u0 = min(t.s0 for t in tiles)
        u1 = max(t.s1 for t in tiles)
        tiles[0].s0, tiles[0].s1 = u0, u1
        tiles[0].first = True
        tiles[-1].last = True
        for t in tiles:
            muls = []
            for qt in range(t.s0 // KT, t.s1 // KT):
                full = sums[t.kt, qc * qt_per_qc + qt]
                if full != KT * KT:          # zero or mixed -> needs pattern
                    muls.append((qt, pattern_id(t.kt, qc * qt_per_qc + qt)))
            t.muls = muls
        plan.append(tiles)
    pats = np.stack(patterns) if patterns else np.zeros((1, KT, KT), np.float16)
    return plan, pats


# ----- TileContext with a codegen-safe exit drain ---------------------------
# The stock kernel-tail drain carries one semaphore wait per engine/queue the
# kernel touched; CoreV3 codegen rejects instructions with more than two
# waits ("Too many sync wait commands").  Split the waits across preceding
# sync-engine nops, two per instruction, so the drain itself needs none.
class _TileContext(tile.TileContext):
    def _drain_and_barrier(self, tick_clock, wait_clock):
        from concourse.vector_clock import ScopedClock
        nc = self.nc
        probe = nc.sync.nop()
        wait_clock.add_sem_waits(
            probe.ins, ScopedClock({None: tick_clock.global_clock}))
        si = probe.ins.sync_info
        waits = list(si.on_wait) if si and si.on_wait else []
        if len(waits) > 1:
            probe.ins.sync_info = mybir.SyncInfo(
                on_wait=waits[:1], on_update=list(si.on_update or []))
            for w in waits[1:]:
                n = nc.sync.nop()
                n.ins.sync_info = mybir.SyncInfo(on_wait=[w], on_update=[])
        nc.sync.drain()
        nc.all_engine_barrier()
        assert self.sems is not None
        popped = nc._tile_sem_poison_stack.pop()
        assert popped is self._sem_poison
        nc.clear_and_free_semaphores(list(self.sems.allocated().values()))
        nc.all_engine_barrier()


# The same wait-count limit applies to ordinary engine instructions under
# this walrus build, so after the program is fully built, hoist all but one
# wait of every instruction onto preceding same-engine no-ops.
def _legalize_waits(nc, limit=1):
    for bb in nc.main_func.blocks:
        insts = list(bb.instructions)
        out = []
        for inst in insts:
            si = inst.sync_info
            waits = list(si.on_wait) if si and si.on_wait else []
            if len(waits) > limit:
                for w in waits[:-limit]:
                    nop = mybir.InstNoOp(
                        name=nc.get_next_instruction_name(), ins=[], outs=[])
                    nop.engine = inst.engine
                    nop.sync_info = mybir.SyncInfo(on_wait=[w], on_update=[])
                    nc.register_instruction(nop, overwrite=True)
                    out.append(nop)
                inst.sync_info = mybir.SyncInfo(
                    on_wait=waits[-limit:],
                    on_update=list(si.on_update or []))
            out.append(inst)
        bb.instructions = out


# ----- the bass program -----------------------------------------------------
def build_program(plan, n_pat):
    nc = bass.Bass(num_devices=NCORES)

    xq_dr = nc.dram_tensor("xq", [NQC, P, NFT, QC], F16, kind="ExternalInput")
    xk_dr = nc.dram_tensor("xk", [NQC, P, NFT, QC], F16, kind="ExternalInput")
    xv_dr = nc.dram_tensor("xv", [NQC, P, NFT, QC], F16, kind="ExternalInput")
    wq_dr = nc.dram_tensor("wq", [P, NFT, EL], F16, kind="ExternalInput")
    wk_dr = nc.dram_tensor("wk", [P, NFT, EL], F16, kind="ExternalInput")
    wv_dr = nc.dram_tensor("wv", [P, NFT, EL], F16, kind="ExternalInput")
    wo_dr = nc.dram_tensor("wo", [P, 2, D], F16, kind="ExternalInput")
    bq_dr = nc.dram_tensor("bq", [P, 2], F32, kind="ExternalInput")
    bk_dr = nc.dram_tensor("bk", [P, 2], F32, kind="ExternalInput")
    pats_dr = nc.dram_tensor("pats", [P, n_pat, KT], F16, kind="ExternalInput")
    yT_dr = nc.dram_tensor("yT", [NQC, P, NMT, QC], F16, kind="ExternalOutput")

    with ExitStack() as ctx:
        tc = ctx.enter_context(_TileContext(nc))
        singles = ctx.enter_context(tc.tile_pool(name="singles", bufs=1))

        # --- persistent SBUF state + input DMAs (priority order) ---
        wq_sb = singles.tile([P, NFT, EL], F16)
        wk_sb = singles.tile([P, NFT, EL], F16)
        wv_sb = singles.tile([P, NFT, EL], F16)
        wo_sb = singles.tile([P, 2, D], F16)
        bq_sb = singles.tile([P, 2], F32)
        bk_sb = singles.tile([P, 2], F32)
        pat_sb = singles.tile([P, n_pat, KT], F16)

        xin = ctx.enter_context(tc.tile_pool(name="xin", bufs=9))
        xch = {}

        def emit_x_dma(qc):
            ts = {}
            for nm, dr in (("q", xq_dr), ("k", xk_dr), ("v", xv_dr)):
                tl = xin.tile([P, NFT, QC], F16, tag="xch", name=f"x{nm}{qc}")
                # two half-DMAs so the first projection matmuls can start
                # after half the chunk has landed
                nc.sync.dma_start(out=tl[:, 0:NFT // 2, :],
                                  in_=dr[qc, :, 0:NFT // 2, :])
                nc.sync.dma_start(out=tl[:, NFT // 2:, :],
                                  in_=dr[qc, :, NFT // 2:, :])
                ts[nm] = tl
            xch[qc] = ts

        # DMA priority order: what the first projection units consume first.
        # The Q path loads on the sync ring, the K/V path in parallel on the
        # scalar engine's hwdge ring, halving the startup feed time.
        nc.sync.dma_start(out=bq_sb[:], in_=bq_dr[:])
        nc.sync.dma_start(out=bk_sb[:], in_=bk_dr[:])
        nc.sync.dma_start(out=wq_sb[:], in_=wq_dr[:])
        ts0 = {}
        for nm, dr, eng, wdma, wdr in (("q", xq_dr, nc.sync, None, None),
                                       ("k", xk_dr, nc.scalar, wk_sb, wk_dr),
                                       ("v", xv_dr, nc.scalar, wv_sb, wv_dr)):
            if wdma is not None:
                eng.dma_start(out=wdma[:], in_=wdr[:])
            tl = xin.tile([P, NFT, QC], F16, tag="xch", name=f"x{nm}0")
            eng.dma_start(out=tl[:, 0:NFT // 2, :],
                          in_=dr[0, :, 0:NFT // 2, :])
            eng.dma_start(out=tl[:, NFT // 2:, :],
                          in_=dr[0, :, NFT // 2:, :])
            ts0[nm] = tl
        xch[0] = ts0
        nc.sync.dma_start(out=pat_sb[:], in_=pats_dr[:])
        emit_x_dma(1)
        nc.sync.dma_start(out=wo_sb[:], in_=wo_dr[:])

        Qt = singles.tile([P, 2, S], F16)     # [dim-in-pair, et, t]
        Kt = singles.tile([P, 2, S], F16)
        Vaug = singles.tile([P, NKT, HL, DK + 1], F16)  # [t-in-ktile, kt, h, e|1]
        nc.vector.memset(Vaug[:, :, :, DK:DK + 1], 1.0)
        ones64 = singles.tile([P, DK], F16)   # bcast stationary, row 64 only
        nc.vector.memset(ones64[DK:DK + 1, :], 1.0)

        # touch Exp+Ln early so the activation table loads during the first
        # projection instead of stalling the first softmax
        warm = singles.tile([P, 1], F32)
        nc.scalar.activation(out=warm[0:1, :], in_=bq_sb[0:1, 0:1],
                             func=mybir.ActivationFunctionType.Exp)
        nc.scalar.activation(out=warm[0:1, :], in_=warm[0:1, :],
                             func=mybir.ActivationFunctionType.Ln)

        # --- pools ---
        spool = ctx.enter_context(tc.tile_pool(name="spool", bufs=2,
                                               space="PSUM"))
        avp = ctx.enter_context(tc.tile_pool(name="avp", bufs=2, space="PSUM"))
        gemm = ctx.enter_context(tc.tile_pool(name="gemm", bufs=2,
                                              space="PSUM"))
        ptp = ctx.enter_context(tc.tile_pool(name="ptp", bufs=4))
        xtp = ctx.enter_context(tc.tile_pool(name="xtp", bufs=2))
        rsp = ctx.enter_context(tc.tile_pool(name="rsp", bufs=2))
        cpp = ctx.enter_context(tc.tile_pool(name="cpp", bufs=2))
        tpp = ctx.enter_context(tc.tile_pool(name="tpp", bufs=2))
        ysp = ctx.enter_context(tc.tile_pool(name="ysp", bufs=2))
        dbp = ctx.enter_context(tc.tile_pool(name="dbp", bufs=2,
                                             space="DRAM"))

        # --- projection units (6 per chunk) -------------------------------
        def proj_unit_qk(qc, which, et):
            w_sb, b_sb, dst = ((wq_sb, bq_sb, Qt) if which == "q"
                               else (wk_sb, bk_sb, Kt))
            x_ch = xch[qc][which]
            tsl = slice(qc * QC, (qc + 1) * QC)
            ps = gemm.tile([P, QC], F32, tag="gm", name=f"p{which}{qc}{et}")
            for ft in range(NFT):
                nc.tensor.matmul(
                    ps[:],
                    lhsT=w_sb[:, ft, et * P:(et + 1) * P],
                    rhs=x_ch[:, ft, :],
                    start=(ft == 0), stop=(ft == NFT - 1))
            # bias add + f16 convert on DVE (the scalar engine is the
            # attention-phase bottleneck, DVE has slack)
            nc.vector.tensor_scalar_add(
                out=dst[:, et, tsl], in0=ps[:], scalar1=b_sb[:, et:et + 1])

        def proj_unit_v(qc, h2):
            x_ch = xch[qc]["v"]
            ps = gemm.tile([P, 2, EL], F32, tag="gm", name=f"pv{qc}{h2}")
            for i in range(2):
                tt = 2 * h2 + i
                for ft in range(NFT):
                    nc.tensor.matmul(
                        ps[:, i, :],
                        lhsT=x_ch[:, ft, tt * KT:(tt + 1) * KT],
                        rhs=wv_sb[:, ft, :],
                        start=(ft == 0), stop=(ft == NFT - 1))
            for i in range(2):
                ktg = qc * (QC // KT) + 2 * h2 + i
                # all 4 heads in one strided copy (dst skips the ones col)
                nc.vector.tensor_copy(
                    out=Vaug[:, ktg, :, 0:DK],
                    in_=ps[:, i, :].rearrange("p (h e) -> p h e", h=HL))

        def proj_units(qc):
            return [lambda qc=qc: proj_unit_qk(qc, "q", 0),
                    lambda qc=qc: proj_unit_qk(qc, "q", 1),
                    lambda qc=qc: proj_unit_qk(qc, "k", 0),
                    lambda qc=qc: proj_unit_qk(qc, "k", 1),
                    lambda qc=qc: proj_unit_v(qc, 0),
                    lambda qc=qc: proj_unit_v(qc, 1)]

        # --- output-projection units (8 + flush per chunk) ----------------
        def y_units(qc, xTt, tail=False):
            ys = ysp.tile([P, NMT, QC], F16, tag="ys", name=f"ys{qc}")

            def unit(mt):
                yp = gemm.tile([P, QC], F32, tag="gm", name=f"yp{qc}{mt}")
                for ct in range(2):
                    nc.tensor.matmul(
                        yp[:],
                        lhsT=wo_sb[:, ct, mt * P:(mt + 1) * P],
                        rhs=xTt[:, ct, :],
                        start=(ct == 0), stop=(ct == 1))
                # in the tail the scalar engine is idle: alternate engines
                # so the two psum banks drain twice as fast, and stream each
                # mt tile out as soon as it converts
                if tail and mt % 2 == 1:
                    nc.scalar.activation(
                        out=ys[:, mt, :], in_=yp[:],
                        func=mybir.ActivationFunctionType.Copy)
                else:
                    nc.vector.tensor_copy(out=ys[:, mt, :], in_=yp[:])
                if tail:
                    nc.sync.dma_start(out=yT_dr[qc, :, mt, :],
                                      in_=ys[:, mt, :])

            units = [lambda mt=mt: unit(mt) for mt in range(NMT)]
            if not tail:
                units.append(lambda: nc.sync.dma_start(out=yT_dr[qc],
                                                       in_=ys[:]))
            return units

        # --- attention ----------------------------------------------------
        def attention_pair(qc, hp, xTt, mid_units, last=False):
            """scores/exp/AV stream for head pair hp of chunk qc, then the
            normalize chain.  mid_units: tensor-engine filler emitted
            between the AV tail and the normalize broadcasts."""
            et = hp
            tiles = plan[qc]
            av = [avp.tile([P, QC], F32, tag="av", name=f"av{qc}{hp}{hh}")
                  for hh in range(2)]
            pts = []

            def emit_av(ti, t):
                for hh in range(2):
                    nc.tensor.matmul(
                        av[hh][0:DK + 1, t.s0:t.s1],
                        lhsT=Vaug[:, t.kt, 2 * hp + hh, :],
                        rhs=pts[ti][:, hh, t.s0:t.s1],
                        start=t.first, stop=t.last,
                        skip_group_check=True)

            for ti, t in enumerate(tiles):
                pt = ptp.tile([P, 2, QC], F16, tag="pt", name=f"pt{qc}{hp}{ti}")
                pts.append(pt)
                ps = spool.tile([P, 2, QC], F32, tag="s",
                                name=f"s{qc}{hp}{ti}")
                for hh in range(2):
                    po = hh * DK
                    nc.tensor.matmul(
                        ps[:, hh, t.s0:t.s1],
                        lhsT=Kt[po:po + DK, et, t.kt * KT:(t.kt + 1) * KT],
                        rhs=Qt[po:po + DK, et,
                               qc * QC + t.s0:qc * QC + t.s1],
                        start=True, stop=True)
                if ti > 0:
                    emit_av(ti - 1, tiles[ti - 1])
                # one exp for both heads (strided 3D AP over the pair dim)
                nc.scalar.activation(
                    out=pts[ti][:, :, t.s0:t.s1],
                    in_=ps[:, :, t.s0:t.s1],
                    func=mybir.ActivationFunctionType.Exp,
                    scale=float(SCALE))
                for hh in range(2):
                    for qt, pid in t.muls:
                        sl = slice(qt * KT, (qt + 1) * KT)
                        nc.vector.tensor_tensor(
                            out=pts[ti][:, hh, sl], in0=pts[ti][:, hh, sl],
                            in1=pat_sb[:, pid, :], op=mybir.AluOpType.mult)
            emit_av(len(tiles) - 1, tiles[-1])

            # normalize: 1/denom = exp(-ln(denom)) on the scalar engine
            # ([1,512] rows, same Exp/Ln table), broadcast across the 64
            # head dims, then multiply the copied-out head values.  The
            # broadcast is a DRAM round-trip (latency hides: xTt is only
            # consumed one chunk later) except for the very last pair,
            # where latency matters and a ones-row PE matmul is used.
            rc, cp = [], []
            for hh in range(2):
                ln = rsp.tile([P, QC], F32, tag="ln", name=f"ln{qc}{hp}{hh}")
                r = rsp.tile([P, QC], F16, tag="r", name=f"r{qc}{hp}{hh}")
                nc.scalar.activation(out=ln[DK:DK + 1, :],
                                     in_=av[hh][DK:DK + 1, :],
                                     func=mybir.ActivationFunctionType.Ln)
                nc.scalar.activation(out=r[DK:DK + 1, :],
                                     in_=ln[DK:DK + 1, :],
                                     func=mybir.ActivationFunctionType.Exp,
                                     scale=-1.0)
                c = cpp.tile([P, QC], F16, tag="c", name=f"c{qc}{hp}{hh}")
                nc.vector.tensor_copy(out=c[0:DK, :], in_=av[hh][0:DK, :])
                rc.append(r)
                cp.append(c)
            bcs = []
            if not last:
                for hh in range(2):
                    dnb = dbp.tile([1, QC], F16, tag="dnb",
                                   name=f"dnb{qc}{hp}{hh}")
                    nc.sync.dma_start(out=dnb[:], in_=rc[hh][DK:DK + 1, :])
                    bc = rsp.tile([P, QC], F16, tag="bc",
                                  name=f"bc{qc}{hp}{hh}")
                    nc.sync.dma_start(out=bc[0:DK, :],
                                      in_=dnb[0:1, :].partition_broadcast(DK))
                    bcs.append(bc)

            for u in mid_units:
                u()

            for hh in range(2):
                if last:
                    rb = gemm.tile([P, QC], F32, tag="gm",
                                   name=f"rb{qc}{hp}{hh}")
                    nc.tensor.matmul(rb[0:DK, :],
                                     lhsT=ones64[DK:DK + 1, :],
                                     rhs=rc[hh][DK:DK + 1, :],
                                     start=True, stop=True)
                    bc_ap = rb[0:DK, :]
                else:
                    bc_ap = bcs[hh][0:DK, :]
                if hh == 0:
                    nc.vector.tensor_tensor(
                        out=xTt[0:DK, et, :], in0=cp[hh][0:DK, :],
                        in1=bc_ap, op=mybir.AluOpType.mult)
                else:
                    tmp = tpp.tile([P, QC], F16, tag="tm", name=f"tm{qc}{hp}")
                    nc.vector.tensor_tensor(
                        out=tmp[0:DK, :], in0=cp[hh][0:DK, :],
                        in1=bc_ap, op=mybir.AluOpType.mult)
                    nc.sync.dma_start(out=xTt[DK:P, et, :], in_=tmp[0:DK, :])

        # --- the pipeline -------------------------------------------------
        for u in proj_units(0):
            u()

        prev_xTt = None
        for qc in range(NQC):
            xTt = xtp.tile([P, 2, QC], F16, tag="xT", name=f"xT{qc}")
            yu = y_units(qc - 1, prev_xTt) if prev_xTt is not None else []
            pu = proj_units(qc + 1) if qc + 1 < NQC else []
            # tensor-engine filler: y units of the previous chunk and the
            # next chunk's projections, split so both head pairs' normalize
            # latency is hidden.
            fill = yu + pu
            attention_pair(qc, 0, xTt, fill[:4])
            attention_pair(qc, 1, xTt, fill[4:7], last=(qc == NQC - 1))
            for u in fill[7:]:
                u()
            if qc + 2 < NQC:
                emit_x_dma(qc + 2)
            prev_xTt = xTt
        for u in y_units(NQC - 1, prev_xTt, tail=True):
            u()

    _legalize_waits(nc)
    return nc


# ----- SPMD runner ----------------------------------------------------------
# run_bass_kernel_spmd's axon path lowers through jax.jit(shard_map(...)),
# which this jax version emits as `call`-indirect HLO that the bass_exec
# compile hook rejects, and a single 8-replica launch isn't reachable from
# here.  Instead: one single-device jit per core (clean single-computation
# HLO), dispatched asynchronously on all 8 cores.  The NEFF is memoized by
# HLO bytes so walrus runs once, not 8 times.
_NEFF_MEMO = {}


def _install_memo_hook():
    import libneuronxla
    from concourse.bass2jax import install_neuronx_cc_hook

    install_neuronx_cc_hook()
    inner = libneuronxla.neuronx_cc
    if getattr(inner, "_is_memo_hook", False):
        return

    def memo_hook(code, code_format, platform_version, file_prefix):
        import hashlib
        key = hashlib.sha256(bytes(code)).hexdigest()
        if key not in _NEFF_MEMO:
            _NEFF_MEMO[key] = inner(code, code_format, platform_version,
                                    file_prefix)
        return _NEFF_MEMO[key]

    memo_hook._is_memo_hook = True
    libneuronxla.neuronx_cc = memo_hook


def run_spmd(nc, in_maps):
    import jax
    from concourse.bass2jax import _bass_exec_p

    _install_memo_hook()
    n_cores = len(in_maps)
    partition_name = (nc.partition_id_tensor.name
                      if nc.partition_id_tensor is not None else None)
    in_names, out_names, out_avals = [], [], []
    for alloc in nc.m.functions[0].allocations:
        if not isinstance(alloc, mybir.MemoryLocationSet):
            continue
        name = alloc.memorylocations[0].name
        if alloc.kind == "ExternalInput":
            if name != partition_name:
                in_names.append(name)
        elif alloc.kind == "ExternalOutput":
            out_names.append(name)
            out_avals.append(jax.core.ShapedArray(
                tuple(alloc.tensor_shape), mybir.dt.np(alloc.dtype)))
    bind_in_names = tuple(in_names +
                          ([partition_name] if partition_name else []))

    def _body(*args):
        return tuple(_bass_exec_p.bind(
            *args, out_avals=tuple(out_avals), in_names=bind_in_names,
            out_names=tuple(out_names), lowering_input_output_aliases=(),
            sim_require_finite=True, sim_require_nnan=True, nc=nc))

    devices = jax.devices()[:n_cores]
    f = jax.jit(_body)
    futs = []
    for c in range(n_cores):
        args = [jax.device_put(np.asarray(in_maps[c][nm]), devices[c])
                for nm in in_names]
        if partition_name:
            args.append(jax.device_put(np.array([[c]], np.uint32), devices[c]))
        futs.append(f(*args))
    return [{nm: np.asarray(futs[c][i]) for i, nm in enumerate(out_names)}
            for c in range(n_cores)]


# ----- host wrapper ---------------------------------------------------------
_CACHE = {}


def _get_program(mask):
    key = mask.tobytes()
    if key not in _CACHE:
        plan, pats = _mask_plan(mask)
        nc = build_program(plan, pats.shape[0])
        _CACHE[key] = (nc, pats)
    return _CACHE[key]


def _chunked_xT(x):
    # x [S, D] fp32 -> x^T chunk-contiguous [NQC, P, NFT, QC] fp16
    xt = np.ascontiguousarray(x.T.astype(F16NP))          # [D, S]
    return np.ascontiguousarray(
        xt.reshape(NFT, P, NQC, QC).transpose(2, 1, 0, 3))


def make_in_maps(q, k, v, mask, wq, bq, wk, bk, wv, bv, wo, bo, pats):
    q, k, v = (np.asarray(a, np.float32) for a in (q, k, v))
    pats_t = np.ascontiguousarray(pats.transpose(1, 0, 2))  # [ki, n, qi]
    in_maps = []
    for c in range(NCORES):
        b, g = divmod(c, TP)
        sl = slice(g * EL, (g + 1) * EL)
        in_maps.append({
            "xq": _chunked_xT(q[b]),
            "xk": _chunked_xT(k[b]),
            "xv": _chunked_xT(v[b]),
            "wq": np.ascontiguousarray(
                wq[sl, :].T.astype(F16NP).reshape(NFT, P, EL)
                .transpose(1, 0, 2)),
            "wk": np.ascontiguousarray(
                wk[sl, :].T.astype(F16NP).reshape(NFT, P, EL)
                .transpose(1, 0, 2)),
            "wv": np.ascontiguousarray(
                wv[sl, :].T.astype(F16NP).reshape(NFT, P, EL)
                .transpose(1, 0, 2)),
            "wo": np.ascontiguousarray(
                wo[:, sl].T.astype(F16NP).reshape(2, P, D)
                .transpose(1, 0, 2)),
            "bq": np.ascontiguousarray(bq[sl].reshape(2, P).T),
            "bk": np.ascontiguousarray(bk[sl].reshape(2, P).T),
            "pats": pats_t,
        })
    return in_maps


def assemble_output(results, bv, wo, bo):
    ybias = (np.asarray(bv, np.float64) @ np.asarray(wo, np.float64).T
             + np.asarray(bo, np.float64)).astype(np.float32)
    y = np.empty((B, S, D), np.float32)
    for b in range(B):
        acc = results[b * TP]["yT"].astype(np.float32)
        for g in range(1, TP):
            acc = acc + results[b * TP + g]["yT"]
        # acc [NQC, P, NMT, QC] -> y^T [D, S] -> y [S, D]
        yt = acc.transpose(2, 1, 0, 3).reshape(D, S)
        y[b] = yt.T + ybias[None, :]
    return y


def kernel(q, k, v, mask, wq, bq, wk, bk, wv, bv, wo, bo):
    mask2d = np.asarray(mask).reshape(S, S)
    nc, pats = _get_program(mask2d)
    in_maps = make_in_maps(q, k, v, mask2d, wq, bq, wk, bk, wv, bv, wo, bo,
                           pats)
    return assemble_output(run_spmd(nc, in_maps), bv, wo, bo)
